# revision 17
# baseline (speedup 1.0000x reference)
"""DSH loss kernel for Trainium2 (8 NeuronCores, Bass/Tile).

Math (reference):
    U[ind] = u; Y[ind] = y
    raw[b,n]  = ||u_b||^2 - 2 u_b.U_n + ||U_n||^2          (>= 0 mathematically)
    dist      = max(raw, 0)
    match[b,n]= y_b . Y_n          (integer >= 0)
    m         = (match == 0)       ("mismatch" mask, statistically ~never 1)
    loss1 = mean( (1-m)*0.5*dist + m*0.5*relu(M - dist) )
    loss2 = ALPHA * mean(|1 - sign(u)|)

Decomposition:
    2*B*N*loss1 = S_raw + sum_{m=1} [ relu(M - raw) - raw ]
      S_raw factorizes: N*sum(u_sq) + B*sum(U_sq) - 2*colsum(u).colsum(U)
      -> computed exactly on host in fp64.
    The correction term needs the m==1 pairs. The device computes, for
    every (U-row, batch) pair, x1 = raw + BIG*match in one PSUM
    accumulation group (two bf16 matmuls, contraction dims 66 and 100),
    then one fused elementwise+reduce pass per tile:
        det = relu(T0 - x1)   with T0 chosen so  BIG >> T0 >> max(raw):
    x1 < T0 iff match == 0, so the per-row reduced det is nonzero iff
    that U-row has a match==0 pair -> exact detector. Flagged rows
    (normally none) are corrected exactly on host in fp64.
    The pass alternates between ScalarE (activation w/ accum_out) and
    VectorE (tensor_scalar subtract/min w/ accum_out) to balance engines.

Device tiling per core (shard = 12500 U/Y rows):
    98 tiles of 128 U-rows; stationary = UA/YT tile [K,128],
    moving = augmented uaT [66,512] / BIG*yT [100,512]; PSUM [128,512].
"""

import numpy as np
import ml_dtypes

import concourse.bass as bass
import concourse.mybir as mybir
import concourse.tile as tile
from concourse import bacc
from concourse.bass_utils import run_bass_kernel_spmd

# Problem constants (hardcoded per harness contract)
B = 512
BIT = 64
C = 100
N = 100000
N_CORES = 8
N_SH = N // N_CORES          # 12500
M_MARGIN = 2.0 * BIT         # 128.0
ALPHA = 0.1
BIG = 16384.0                # power of two; BIG*match exact in fp32/bf16
T0 = 8192.0                  # detector threshold: max(raw) << T0 << BIG
KA = BIT + 2                 # augmented contraction dim for the dist matmul
P_TILE = 128                 # U-rows per tile (PSUM partition dim)
F_B = B                      # moving free dim = full batch = 512
ACT_EVERY = 2                # tile t uses ScalarE if t % ACT_EVERY == 0 else DVE

BF16 = ml_dtypes.bfloat16


def _build_program(n_sh: int):
    """v3: K=128 zero-padded operands (full-rate 216ns/MM), super-tiles of
    2x128 U-rows sharing one [128,1024] PSUM tile and ONE fused
    elementwise+accum pass, alternating ScalarE/VectorE."""
    fp32 = mybir.dt.float32
    bf16 = mybir.dt.bfloat16
    nc = bacc.Bacc("TRN2", target_bir_lowering=False)

    n_pad = ((n_sh + 2 * P_TILE - 1) // (2 * P_TILE)) * (2 * P_TILE)
    n_tiles = n_pad // P_TILE
    n_super = n_tiles // 2

    # all operands arrive zero-padded to K=128 rows (full matmul rate +
    # full-bandwidth 128-partition DMA)
    uaT_d = nc.declare_dram_parameter("uaT", [128, B], bf16, isOutput=False)
    ypT_d = nc.declare_dram_parameter("ypT", [128, B], bf16, isOutput=False)
    UA_d = nc.declare_dram_parameter("UA", [128, n_sh], bf16, isOutput=False)
    YT_d = nc.declare_dram_parameter("YT", [128, n_sh], bf16, isOutput=False)
    accD_d = nc.declare_dram_parameter("accD", [128, n_super], fp32, isOutput=True)

    with tile.TileContext(nc) as tc:
        with (
            tc.tile_pool(name="resident", bufs=1) as resident,
            tc.tile_pool(name="scr", bufs=4) as scrp,
            tc.tile_pool(name="psum", bufs=4, space="PSUM") as psump,
        ):
            ua_sb = resident.tile([128, B], bf16, tag="ua")
            yp_sb = resident.tile([128, B], bf16, tag="yp")
            UA_sb = resident.tile([128, n_pad], bf16, tag="UA")
            YT_sb = resident.tile([128, n_pad], bf16, tag="YT")
            accD = resident.tile([128, n_super], fp32, tag="accD")
            bias_t0 = resident.tile([128, 1], fp32, tag="biast0")

            # Moving operands first (tiny, needed by every matmul); then the
            # gallery slices, small-first so tile 0 is ready ASAP.
            # UA on the sync queue, YT on the gpsimd queue -> parallel DMA.
            # moving operands on the scalar/vector queues so the gallery
            # slices start streaming on sync/gpsimd at the same time
            nc.scalar.dma_start(ua_sb[:], uaT_d[:])
            nc.scalar.dma_start(yp_sb[:], ypT_d[:])
            s = 0
            widths = [128, 128, 256, 512] + [1024] * 12
            for w in widths:
                if s >= n_sh:
                    break
                w = min(w, n_sh - s)
                nc.sync.dma_start(UA_sb[:, s : s + w], UA_d[:, s : s + w])
                nc.gpsimd.dma_start(YT_sb[:, s : s + w], YT_d[:, s : s + w])
                s += w
            if s < n_sh:
                nc.sync.dma_start(UA_sb[:, s:n_sh], UA_d[:, s:])
                nc.gpsimd.dma_start(YT_sb[:, s:n_sh], YT_d[:, s:])

            # Column padding: UA pad cols = 0; YT pad cols = 1.0 so the
            # padded "gallery rows" match everything -> never flagged.
            if n_pad > n_sh:
                nc.vector.memset(UA_sb[:, n_sh:], 0.0)
                nc.vector.memset(YT_sb[:, n_sh:], 1.0)
            nc.vector.memset(bias_t0[:], T0)
            nc.vector.memset(accD[:], 0.0)

            for sidx in range(n_super):
                x1 = psump.tile([P_TILE, 1024], fp32, tag="x1")
                for h in (0, 1):
                    t = 2 * sidx + h
                    ns = slice(t * P_TILE, (t + 1) * P_TILE)
                    half = x1[:, h * 512 : (h + 1) * 512]
                    nc.tensor.matmul(
                        half, lhsT=UA_sb[:, ns], rhs=ua_sb[:],
                        start=True, stop=False,
                    )
                    nc.tensor.matmul(
                        half, lhsT=YT_sb[:, ns], rhs=yp_sb[:],
                        start=False, stop=True,
                    )

                col = accD[:, sidx : sidx + 1]
                if sidx % ACT_EVERY == 0:
                    scr = scrp.tile([P_TILE, 1024], bf16, tag="scrA")
                    # relu(T0 - x1); accum col > 0 iff some match==0 here
                    nc.scalar.activation(
                        scr[:], x1[:],
                        mybir.ActivationFunctionType.Relu,
                        bias=bias_t0[:], scale=-1.0,
                        accum_out=col,
                    )
                else:
                    scr = scrp.tile([P_TILE, 1024], bf16, tag="scrB")
                    # min(x1 - T0, 0); accum col < 0 iff some match==0 here
                    nc.vector.tensor_scalar(
                        scr[:], x1[:], T0, 0.0,
                        mybir.AluOpType.subtract, mybir.AluOpType.min,
                        accum_out=col,
                    )

            nc.sync.dma_start(accD_d[:], accD[:])

    nc.finalize()
    return nc, n_super


def _prep_host(u, y, ind, U, Y):
    """Scatter + device arrays (bf16) + fp64 base sum."""
    u = np.asarray(u, dtype=np.float32)
    y = np.asarray(y, dtype=np.float32)
    ind = np.asarray(ind).astype(np.int64)
    U2 = np.array(U, dtype=np.float32, copy=True)
    Y2 = np.array(Y, dtype=np.float32, copy=True)
    U2[ind] = u
    Y2[ind] = y

    u64 = u.astype(np.float64)
    U64 = U2.astype(np.float64)
    u_sq64 = (u64 * u64).sum(axis=1)            # [B]
    U_sq64 = (U64 * U64).sum(axis=1)            # [N]
    s_raw = (
        N * u_sq64.sum()
        + B * U_sq64.sum()
        - 2.0 * (u64.sum(axis=0) @ U64.sum(axis=0))
    )

    # K=128 zero-padded operands (rows: 64 dims | U_sq/1 | 1/u_sq | zeros)
    uaT = np.zeros((128, B), dtype=BF16)
    uaT[:BIT] = (-2.0 * u).T.astype(BF16)
    uaT[BIT] = BF16(1.0)
    uaT[BIT + 1] = u_sq64.astype(BF16)
    UA = np.zeros((128, N), dtype=BF16)
    UA[:BIT] = U2.T.astype(BF16)
    UA[BIT] = U_sq64.astype(BF16)
    UA[BIT + 1] = BF16(1.0)

    ypT = np.zeros((128, B), dtype=BF16)
    ypT[:C] = (BIG * y).T.astype(BF16)
    YT = np.zeros((128, N), dtype=BF16)
    YT[:C] = Y2.T.astype(BF16)

    return u, y, U2, Y2, uaT, UA, ypT, YT, s_raw


_PROG_CACHE = {}


def _get_program():
    key = ("v2", N_SH)
    if key not in _PROG_CACHE:
        _PROG_CACHE[key] = _build_program(N_SH)
    return _PROG_CACHE[key]


def kernel(u, y, ind, U, Y):
    u, y, U2, Y2, uaT, UA, ypT, YT, s_raw = _prep_host(u, y, ind, U, Y)

    nc, n_super = _get_program()
    in_maps = []
    for c in range(N_CORES):
        ns = slice(c * N_SH, (c + 1) * N_SH)
        in_maps.append({
            "uaT": uaT,
            "ypT": ypT,
            "UA": np.ascontiguousarray(UA[:, ns]),
            "YT": np.ascontiguousarray(YT[:, ns]),
        })

    res = run_bass_kernel_spmd(nc, in_maps, list(range(N_CORES)))
    results = res.results

    corr = 0.0
    for c in range(N_CORES):
        accD = np.asarray(results[c]["accD"], dtype=np.float64)
        flagged = np.argwhere(np.abs(accD) > 0.5)
        for p, sidx in flagged:
            # super-tile covers two 128-row tiles sharing partition p
            for h in (0, 1):
                n_loc = (2 * sidx + h) * P_TILE + p
                if n_loc >= N_SH:
                    continue  # padded column
                n_glob = c * N_SH + n_loc
                match = y.astype(np.float64) @ Y2[n_glob].astype(np.float64)
                zrows = np.nonzero(match == 0.0)[0]
                for b in zrows:
                    d = u[b].astype(np.float64) - U2[n_glob].astype(np.float64)
                    raw = float(d @ d)
                    corr += max(M_MARGIN - raw, 0.0) - raw

    total2 = s_raw + corr
    loss1 = 0.5 * total2 / (B * N)

    sign_u = np.sign(u)
    loss2 = ALPHA * np.abs(1.0 - sign_u).mean(dtype=np.float64)

    return np.array(loss1 + loss2, dtype=np.float32)


# revision 20
# speedup vs baseline: 1.0061x; 1.0061x over previous
"""DSH loss kernel for Trainium2 (8 NeuronCores, Bass/Tile).

Math (reference):
    U[ind] = u; Y[ind] = y
    raw[b,n]  = ||u_b||^2 - 2 u_b.U_n + ||U_n||^2          (>= 0 mathematically)
    dist      = max(raw, 0)
    match[b,n]= y_b . Y_n          (integer >= 0)
    m         = (match == 0)       ("mismatch" mask, statistically ~never 1)
    loss1 = mean( (1-m)*0.5*dist + m*0.5*relu(M - dist) )
    loss2 = ALPHA * mean(|1 - sign(u)|)

Decomposition:
    2*B*N*loss1 = S_raw + sum_{m=1} [ relu(M - raw) - raw ]
      S_raw factorizes: N*sum(u_sq) + B*sum(U_sq) - 2*colsum(u).colsum(U)
      -> computed exactly on host in fp64.
    The correction term needs the m==1 pairs. The device computes, for
    every (U-row, batch) pair, x1 = raw + BIG*match in one PSUM
    accumulation group (two bf16 matmuls, contraction dims 66 and 100),
    then one fused elementwise+reduce pass per tile:
        det = relu(T0 - x1)   with T0 chosen so  BIG >> T0 >> max(raw):
    x1 < T0 iff match == 0, so the per-row reduced det is nonzero iff
    that U-row has a match==0 pair -> exact detector. Flagged rows
    (normally none) are corrected exactly on host in fp64.
    The pass alternates between ScalarE (activation w/ accum_out) and
    VectorE (tensor_scalar subtract/min w/ accum_out) to balance engines.

Device tiling per core (shard = 12500 U/Y rows):
    98 tiles of 128 U-rows; stationary = UA/YT tile [K,128],
    moving = augmented uaT [66,512] / BIG*yT [100,512]; PSUM [128,512].
"""

import numpy as np
import ml_dtypes

import concourse.bass as bass
import concourse.mybir as mybir
import concourse.tile as tile
from concourse import bacc
from concourse.bass_utils import run_bass_kernel_spmd

# Problem constants (hardcoded per harness contract)
B = 512
BIT = 64
C = 100
N = 100000
N_CORES = 8
N_SH = N // N_CORES          # 12500
M_MARGIN = 2.0 * BIT         # 128.0
ALPHA = 0.1
BIG = 16384.0                # power of two; BIG*match exact in fp32/bf16
T0 = 8192.0                  # detector threshold: max(raw) << T0 << BIG
KA = BIT + 2                 # augmented contraction dim for the dist matmul
P_TILE = 128                 # U-rows per tile (PSUM partition dim)
F_B = B                      # moving free dim = full batch = 512
ACT_EVERY = 2                # tile t uses ScalarE if t % ACT_EVERY == 0 else DVE

BF16 = ml_dtypes.bfloat16


def _build_program(n_sh: int):
    """v3: K=128 zero-padded operands (full-rate 216ns/MM), super-tiles of
    2x128 U-rows sharing one [128,1024] PSUM tile and ONE fused
    elementwise+accum pass, alternating ScalarE/VectorE."""
    fp32 = mybir.dt.float32
    bf16 = mybir.dt.bfloat16
    nc = bacc.Bacc("TRN2", target_bir_lowering=False)

    n_pad = ((n_sh + 2 * P_TILE - 1) // (2 * P_TILE)) * (2 * P_TILE)
    n_tiles = n_pad // P_TILE
    n_super = n_tiles // 2

    # all operands arrive zero-padded to K=128 rows (full matmul rate +
    # full-bandwidth 128-partition DMA)
    uaT_d = nc.declare_dram_parameter("uaT", [128, B], bf16, isOutput=False)
    ypT_d = nc.declare_dram_parameter("ypT", [128, B], bf16, isOutput=False)
    UA_d = nc.declare_dram_parameter("UA", [128, n_sh], bf16, isOutput=False)
    YT_d = nc.declare_dram_parameter("YT", [128, n_sh], bf16, isOutput=False)
    accD_d = nc.declare_dram_parameter("accD", [128, n_super], fp32, isOutput=True)

    with tile.TileContext(nc) as tc:
        with (
            tc.tile_pool(name="resident", bufs=1) as resident,
            tc.tile_pool(name="scr", bufs=4) as scrp,
            tc.tile_pool(name="psum", bufs=4, space="PSUM") as psump,
        ):
            ua_sb = resident.tile([128, B], bf16, tag="ua")
            yp_sb = resident.tile([128, B], bf16, tag="yp")
            UA_sb = resident.tile([128, n_pad], bf16, tag="UA")
            YT_sb = resident.tile([128, n_pad], bf16, tag="YT")
            accD = resident.tile([128, n_super], fp32, tag="accD")
            bias_t0 = resident.tile([128, 1], fp32, tag="biast0")

            # Moving operands first (tiny, needed by every matmul); then the
            # gallery slices, small-first so tile 0 is ready ASAP.
            # UA on the sync queue, YT on the gpsimd queue -> parallel DMA.
            # moving operands on the scalar/vector queues so the gallery
            # slices start streaming on sync/gpsimd at the same time
            nc.scalar.dma_start(ua_sb[:], uaT_d[:])
            nc.sync.dma_start(yp_sb[:, :256], ypT_d[:, :256])
            nc.gpsimd.dma_start(yp_sb[:, 256:], ypT_d[:, 256:])
            s = 0
            widths = [256, 256, 512] + [1024] * 12
            for w in widths:
                if s >= n_sh:
                    break
                w = min(w, n_sh - s)
                nc.sync.dma_start(UA_sb[:, s : s + w], UA_d[:, s : s + w])
                nc.gpsimd.dma_start(YT_sb[:, s : s + w], YT_d[:, s : s + w])
                s += w
            if s < n_sh:
                nc.sync.dma_start(UA_sb[:, s:n_sh], UA_d[:, s:])
                nc.gpsimd.dma_start(YT_sb[:, s:n_sh], YT_d[:, s:])

            # Column padding: UA pad cols = 0; YT pad cols = 1.0 so the
            # padded "gallery rows" match everything -> never flagged.
            if n_pad > n_sh:
                nc.vector.memset(UA_sb[:, n_sh:], 0.0)
                nc.vector.memset(YT_sb[:, n_sh:], 1.0)
            nc.vector.memset(bias_t0[:], T0)
            nc.vector.memset(accD[:], 0.0)

            for sidx in range(n_super):
                x1 = psump.tile([P_TILE, 1024], fp32, tag="x1")
                for h in (0, 1):
                    t = 2 * sidx + h
                    ns = slice(t * P_TILE, (t + 1) * P_TILE)
                    half = x1[:, h * 512 : (h + 1) * 512]
                    nc.tensor.matmul(
                        half, lhsT=UA_sb[:, ns], rhs=ua_sb[:],
                        start=True, stop=False,
                    )
                    nc.tensor.matmul(
                        half, lhsT=YT_sb[:, ns], rhs=yp_sb[:],
                        start=False, stop=True,
                    )

                col = accD[:, sidx : sidx + 1]
                if sidx % ACT_EVERY == 0:
                    scr = scrp.tile([P_TILE, 1024], bf16, tag="scrA")
                    # relu(T0 - x1); accum col > 0 iff some match==0 here
                    nc.scalar.activation(
                        scr[:], x1[:],
                        mybir.ActivationFunctionType.Relu,
                        bias=bias_t0[:], scale=-1.0,
                        accum_out=col,
                    )
                else:
                    scr = scrp.tile([P_TILE, 1024], bf16, tag="scrB")
                    # min(x1 - T0, 0); accum col < 0 iff some match==0 here
                    nc.vector.tensor_scalar(
                        scr[:], x1[:], T0, 0.0,
                        mybir.AluOpType.subtract, mybir.AluOpType.min,
                        accum_out=col,
                    )

            nc.sync.dma_start(accD_d[:], accD[:])

    nc.finalize()
    return nc, n_super


def _prep_host(u, y, ind, U, Y):
    """Scatter + device arrays (bf16) + fp64 base sum."""
    u = np.asarray(u, dtype=np.float32)
    y = np.asarray(y, dtype=np.float32)
    ind = np.asarray(ind).astype(np.int64)
    U2 = np.array(U, dtype=np.float32, copy=True)
    Y2 = np.array(Y, dtype=np.float32, copy=True)
    U2[ind] = u
    Y2[ind] = y

    u64 = u.astype(np.float64)
    U64 = U2.astype(np.float64)
    u_sq64 = (u64 * u64).sum(axis=1)            # [B]
    U_sq64 = (U64 * U64).sum(axis=1)            # [N]
    s_raw = (
        N * u_sq64.sum()
        + B * U_sq64.sum()
        - 2.0 * (u64.sum(axis=0) @ U64.sum(axis=0))
    )

    # K=128 zero-padded operands (rows: 64 dims | U_sq/1 | 1/u_sq | zeros)
    uaT = np.zeros((128, B), dtype=BF16)
    uaT[:BIT] = (-2.0 * u).T.astype(BF16)
    uaT[BIT] = BF16(1.0)
    uaT[BIT + 1] = u_sq64.astype(BF16)
    UA = np.zeros((128, N), dtype=BF16)
    UA[:BIT] = U2.T.astype(BF16)
    UA[BIT] = U_sq64.astype(BF16)
    UA[BIT + 1] = BF16(1.0)

    ypT = np.zeros((128, B), dtype=BF16)
    ypT[:C] = (BIG * y).T.astype(BF16)
    YT = np.zeros((128, N), dtype=BF16)
    YT[:C] = Y2.T.astype(BF16)

    return u, y, U2, Y2, uaT, UA, ypT, YT, s_raw


def _full_numpy_loss(u, y, U2, Y2):
    """Exact fp64 fallback (blocked); only used if detector preconditions
    fail (non-binary labels / unbounded distances) -- never on spec inputs."""
    total = 0.0
    U64 = U2.astype(np.float64)
    Y64 = Y2.astype(np.float64)
    U_sq = (U64 * U64).sum(axis=1)
    for b0 in range(0, B, 64):
        ub = u[b0 : b0 + 64].astype(np.float64)
        yb = y[b0 : b0 + 64].astype(np.float64)
        dist = np.maximum(
            (ub * ub).sum(1)[:, None] - 2.0 * (ub @ U64.T) + U_sq[None, :], 0.0)
        mism = (yb @ Y64.T) == 0.0
        total += np.where(mism, 0.5 * np.maximum(M_MARGIN - dist, 0.0),
                          0.5 * dist).sum()
    loss1 = total / (B * N)
    loss2 = ALPHA * np.abs(1.0 - np.sign(u)).mean(dtype=np.float64)
    return np.array(loss1 + loss2, dtype=np.float32)


def _detector_preconditions_ok(u, y, U2, Y2):
    if not (((y == 0.0) | (y == 1.0)).all() and ((Y2 == 0.0) | (Y2 == 1.0)).all()):
        return False
    bnd = (np.linalg.norm(u, axis=1).max() + np.linalg.norm(U2, axis=1).max()) ** 2
    return bnd < 0.9 * T0


_PROG_CACHE = {}


def _get_program():
    key = ("v2", N_SH)
    if key not in _PROG_CACHE:
        _PROG_CACHE[key] = _build_program(N_SH)
    return _PROG_CACHE[key]


def kernel(u, y, ind, U, Y):
    u, y, U2, Y2, uaT, UA, ypT, YT, s_raw = _prep_host(u, y, ind, U, Y)

    if not _detector_preconditions_ok(u, y, U2, Y2):
        return _full_numpy_loss(u, y, U2, Y2)

    nc, n_super = _get_program()
    in_maps = []
    for c in range(N_CORES):
        ns = slice(c * N_SH, (c + 1) * N_SH)
        in_maps.append({
            "uaT": uaT,
            "ypT": ypT,
            "UA": np.ascontiguousarray(UA[:, ns]),
            "YT": np.ascontiguousarray(YT[:, ns]),
        })

    res = run_bass_kernel_spmd(nc, in_maps, list(range(N_CORES)))
    results = res.results

    corr = 0.0
    for c in range(N_CORES):
        accD = np.asarray(results[c]["accD"], dtype=np.float64)
        flagged = np.argwhere(np.abs(accD) > 0.5)
        for p, sidx in flagged:
            # super-tile covers two 128-row tiles sharing partition p
            for h in (0, 1):
                n_loc = (2 * sidx + h) * P_TILE + p
                if n_loc >= N_SH:
                    continue  # padded column
                n_glob = c * N_SH + n_loc
                match = y.astype(np.float64) @ Y2[n_glob].astype(np.float64)
                zrows = np.nonzero(match == 0.0)[0]
                for b in zrows:
                    d = u[b].astype(np.float64) - U2[n_glob].astype(np.float64)
                    raw = float(d @ d)
                    corr += max(M_MARGIN - raw, 0.0) - raw

    total2 = s_raw + corr
    loss1 = 0.5 * total2 / (B * N)

    sign_u = np.sign(u)
    loss2 = ALPHA * np.abs(1.0 - sign_u).mean(dtype=np.float64)

    return np.array(loss1 + loss2, dtype=np.float32)
